# revision 4
# baseline (speedup 1.0000x reference)
"""NT-Xent loss kernel for 8 TRN2 NeuronCores (Bass/Tile) — v2.

Math identical to v1 (see kernel_baseline.py): the double-exp CE loss
collapses to loss = mean(exp(m_i/T)) - mean(exp(pos_i/T)) with m_i the
off-diagonal row max of sim = R R^T, computed per row as
    m_i = max(exact max over DVE tiles, mu + ln(sum_A exp(K(s-mu)))/K)
with K=400, mu=0.5 (smoothed-max bias ~2.6e-4).

v2 changes vs the 175 us v1:
  1. fp8e4 DoubleRow matmuls: reps are packed [64, 2, N] (D=128 split
     into two 64-partition halves) and each matmul contracts both halves
     in one pass at 0.5 cycles/output-column — PE time halves to ~55 us
     and the input DMA halves to 2 MB.  fp8 e4m3 quantization of the
     normalized reps perturbs sim by sigma ~ 3e-3 which the host-side
     check puts at 3.3e-3 relative on the final loss (tolerance 2e-2).
  2. PSUM is one [128, 4096] mega-tile split into 3 drain regions
     (1536/1536/1024 f32).  The tile framework tracks hazards at access-
     pattern range granularity, so matmuls fill 512-wide slices while
     DVE (reduce_max) and ACT (exp accum) drain whole regions; 3 regions
     keep both drain engines busy while the PE fills the third.
  3. Drain work is split DVE:ACT ~ 42:58 by a greedy schedule weighted
     by engine rates (DVE 0.96 GHz, ACT 1.2 GHz, + per-instr overhead),
     instead of v1's 50:50 — the drains, not the PE, are the roofline.
"""

import os
import numpy as np

TEMP = 0.07
B = 8192
D = 128
N = 2 * B
NCORES = 8
ROWS_PER_CORE = N // NCORES   # 2048
BLKS = ROWS_PER_CORE // 128   # 16 row-blocks per core

KSCALE = 400.0
MU = 0.50

# PSUM mega-tile layout: 3 regions per 4096-f32 bank space
REGIONS = (1536, 1536, 1024)
RBASE = (0, 1536, 3072)
NREG = len(REGIONS)
REG_PER_BLK = 16384 // 512    # 32 matmuls/block; regions consume 3/3/2 mm

# rates for greedy drain assignment (cols/ns incl per-instr overhead)
_DVE_RATE = 0.96e9
_ACT_RATE = 1.2e9
_DVE_OVH = 163.0   # cycles per drain instr (psum init + seq)
_ACT_OVH = 210.0


def build_schedule():
    """Assign each drain region-instance to DVE ('B') or ACT ('A').

    Returns list over all (block, region-instance) of ('A'|'B', col),
    where col is the output column in sacc (A) or emstage (B), plus the
    final column counts (na, nb).
    """
    # region instances per block: widths cycle 1536,1536,1024 over
    # 16384 cols -> 16384/4096 * 3 = 12 region-instances per block
    per_blk = []
    off = 0
    while off < 16384:
        r = len(per_blk) % NREG
        per_blk.append(REGIONS[r])
        off += REGIONS[r]
    assert off == 16384 and len(per_blk) == 12

    sched = []
    tb = 4656.0 / _DVE_RATE * 1e9  # negeye masks pre-charged to DVE (ns)
    ta = 0.0
    na = nb = 0
    for lm in range(BLKS):
        for w in per_blk:
            dt_b = (w + _DVE_OVH) / _DVE_RATE * 1e9
            dt_a = (w + _ACT_OVH) / _ACT_RATE * 1e9
            if tb + dt_b <= ta + dt_a:
                sched.append(("B", nb))
                tb += dt_b
                nb += 1
            else:
                sched.append(("A", na))
                ta += dt_a
                na += 1
    return sched, na, nb


SCHED, NA, NB = build_schedule()
# columns completed after the first BLKS//2 blocks (for the mid-run DMA)
NA_HALF = sum(1 for e, _ in SCHED[: (BLKS // 2) * 12] if e == "A")
NB_HALF = (BLKS // 2) * 12 - NA_HALF
OUT_LEN = (NA + NB) * 128

_cache = {}


def build_nc():
    import concourse.bacc as bacc
    import concourse.bass as bass
    import concourse.mybir as mybir
    import concourse.tile as tile

    f32 = mybir.dt.float32
    bf16 = mybir.dt.bfloat16
    fp8 = mybir.dt.float8e4
    AF = mybir.ActivationFunctionType
    ALU = mybir.AluOpType
    DR = mybir.MatmulPerfMode.DoubleRow

    nc = bacc.Bacc(
        "TRN2",
        target_bir_lowering=False,
        debug=False,
        num_devices=NCORES,
    )

    # packed fp8 reps: row p holds RT[p, :] ++ RT[p+64, :]
    zt_d = nc.dram_tensor("zt", [64, 2 * N], fp8, kind="ExternalInput").ap()
    negeye_d = nc.dram_tensor("negeye", [128, 128], f32, kind="ExternalInput").ap()
    out_d = nc.dram_tensor("out", [OUT_LEN], f32, kind="ExternalOutput").ap()

    with tile.TileContext(nc) as tc:
        with (
            tc.tile_pool(name="cpool", bufs=1) as cpool,
            tc.tile_pool(name="psum", bufs=1, space=bass.MemorySpace.PSUM) as psumpool,
        ):
            negeye = cpool.tile([128, 128], f32, tag="negeye")
            nc.sync.dma_start(negeye[:], negeye_d[:])

            zt = cpool.tile([64, 2 * N], fp8, tag="zt")
            # 16 moderate DMAs (~128KB each) so matmuls can start early
            NSPLIT = 16
            SC = 2 * N // NSPLIT
            for g in range(NSPLIT):
                nc.sync.dma_start(
                    zt[:, g * SC:(g + 1) * SC],
                    zt_d[:, g * SC:(g + 1) * SC],
                )
            zt3 = zt[:].rearrange("p (i x) -> p i x", i=2)

            kbias = cpool.tile([128, 1], f32, tag="kbias")
            nc.vector.memset(kbias[:], -KSCALE * MU)
            warm = cpool.tile([128, 1], f32, tag="warm")
            nc.scalar.activation(warm[:], kbias[:], AF.Exp)

            emstage = cpool.tile([128, NB], f32, tag="emstage")
            sacc = cpool.tile([128, NA], f32, tag="sacc")
            dumps = [
                cpool.tile([128, 1536], bf16, tag=f"dump{i}", name=f"dump{i}")
                for i in range(3)
            ]
            mega = psumpool.tile([128, 4096], f32, tag="mega")

            E = NB * 128
            HALF = BLKS // 2
            si = 0  # schedule index
            adump = 0
            for lm in range(BLKS):
                lhsT = zt3[:, :, lm * 128:(lm + 1) * 128]
                dcol = lm * 128  # local diag column (always < 2048)

                # walk regions; fill each with 512-wide matmuls then drain
                reg = 0          # region instance within block (0..11)
                col0 = 0         # column offset within the 16384 row
                while col0 < 16384:
                    w = REGIONS[reg % NREG]
                    base = RBASE[reg % NREG]
                    for t in range(w // 512):
                        c = col0 + t * 512
                        nc.tensor.matmul(
                            mega[:, base + t * 512:base + (t + 1) * 512],
                            lhsT,
                            zt3[:, :, c:c + 512],
                            start=True,
                            stop=True,
                            perf_mode=DR,
                        )
                    if col0 <= dcol < col0 + w:
                        p = base + (dcol - col0)
                        nc.vector.tensor_tensor(
                            mega[:, p:p + 128],
                            mega[:, p:p + 128],
                            negeye[:],
                            op=ALU.add,
                        )
                    eng, col = SCHED[si]
                    si += 1
                    if eng == "B":
                        nc.vector.reduce_max(
                            emstage[:, col:col + 1],
                            mega[:, base:base + w],
                            axis=mybir.AxisListType.X,
                        )
                    else:
                        nc.scalar.activation(
                            dumps[adump % 3][:, 0:w],
                            mega[:, base:base + w],
                            AF.Exp,
                            scale=KSCALE,
                            bias=kbias[:],
                            accum_out=sacc[:, col:col + 1],
                        )
                        adump += 1
                    reg += 1
                    col0 += w

                if lm == HALF - 1:
                    nc.sync.dma_start(
                        out_d[0:E].rearrange("(p f) -> p f", f=NB)[:, 0:NB_HALF],
                        emstage[:, 0:NB_HALF],
                    )
                    nc.sync.dma_start(
                        out_d[E:E + NA * 128].rearrange("(p f) -> p f", f=NA)[
                            :, 0:NA_HALF
                        ],
                        sacc[:, 0:NA_HALF],
                    )

            bh = NB_HALF
            ah = NA_HALF
            nc.sync.dma_start(
                out_d[0:E].rearrange("(p f) -> p f", f=NB)[:, bh:NB],
                emstage[:, bh:NB],
            )
            nc.sync.dma_start(
                out_d[E:E + NA * 128].rearrange("(p f) -> p f", f=NA)[:, ah:NA],
                sacc[:, ah:NA],
            )

    nc.compile()
    return nc


def make_in_maps(z_i: np.ndarray, z_j: np.ndarray):
    import ml_dtypes

    Z = np.concatenate([np.asarray(z_i), np.asarray(z_j)], axis=0).astype(np.float32)
    nrm = np.linalg.norm(Z, axis=1, keepdims=True)
    R = (Z / np.maximum(nrm, 1e-12)).astype(np.float32)
    RT8 = np.ascontiguousarray(R.T).astype(ml_dtypes.float8_e4m3fn)  # [128, N]
    negeye = (-99.0 * np.eye(128)).astype(np.float32)
    in_maps = []
    for c in range(NCORES):
        rt = np.roll(RT8, -c * ROWS_PER_CORE, axis=1)
        # pack [64, 2, N]: row p holds [RT[p,:], RT[p+64,:]]
        packed = np.ascontiguousarray(
            np.stack([rt[:64, :], rt[64:, :]], axis=1).reshape(64, 2 * N)
        )
        in_maps.append({"zt": packed, "negeye": negeye})
    return in_maps


def kernel(z_i: np.ndarray, z_j: np.ndarray) -> np.ndarray:
    from concourse.bass_utils import run_bass_kernel_spmd

    if "nc" not in _cache:
        _cache["nc"] = build_nc()
    nc = _cache["nc"]

    in_maps = make_in_maps(z_i, z_j)
    Z = np.concatenate([np.asarray(z_i), np.asarray(z_j)], axis=0).astype(np.float64)
    Rn = Z / np.maximum(np.linalg.norm(Z, axis=1, keepdims=True), 1e-12)
    pos_half = np.sum(Rn[:B] * Rn[B:], axis=1)
    pos_sum = 2.0 * np.sum(np.exp(pos_half / TEMP))
    res = run_bass_kernel_spmd(
        nc,
        in_maps,
        core_ids=list(range(NCORES)),
        trace=bool(int(os.environ.get("NTX_TRACE", "0"))),
    )
    _cache["last_result"] = res

    E = NB * 128
    total = 0.0
    for c in range(NCORES):
        out = res.results[c]["out"].astype(np.float64)
        em = out[0:E].reshape(128, NB)        # [p, b-col]
        sa = out[E:E + NA * 128].reshape(128, NA)
        mB = em.max(axis=1)                   # [p] per-partition exact max
        # rows of this core: partition p of block lm is row lm*128+p;
        # all B-cols/A-cols of a partition belong to the same 16 global
        # rows (one per block) -- regroup per block
        # emstage col ordering follows SCHED per block; since every
        # block's max/sum just combine per (p, lm), split cols by block:
        bcols = [[] for _ in range(BLKS)]
        acols = [[] for _ in range(BLKS)]
        si = 0
        for lm in range(BLKS):
            for _ in range(12):
                eng, col = SCHED[si]
                si += 1
                (bcols if eng == "B" else acols)[lm].append(col)
        for lm in range(BLKS):
            mBlk = em[:, bcols[lm]].max(axis=1) if bcols[lm] else np.full(128, -np.inf)
            sBlk = sa[:, acols[lm]].sum(axis=1) if acols[lm] else 0.0
            sBlk = np.maximum(sBlk, 1e-300)
            m = np.maximum(mBlk, MU + np.log(sBlk) / KSCALE)
            total += np.sum(np.exp(m / TEMP))
    loss = (total - pos_sum) / float(N)
    return np.float32(loss)
